# revision 26
# baseline (speedup 1.0000x reference)
"""Trainium2 Bass kernel for nn_Attention_4080218931831 (sparse_attention).

Computes, for each batch b:
    q = s_b @ Qw           [512, 32]
    k = s_b @ Kw           [512, 32]
    scores = q @ k^T       [512, 512]
    att = scores^2 * G_b
    out = att / (sum(att, axis=2, keepdims=True) + 0.001)

Algebraic refactor: scores = s_b @ (Qw @ Kw^T) @ s_b^T = s_b @ t_b where
t_b = A @ s_b^T and A = Qw @ Kw^T is [10, 10].  A and t are precomputed on
the host in float64 (0.06% of total FLOPs); the dominant [512,10]x[10,512]
matmul per batch runs on the PE.

Precision strategy (the harness gate is rel_err < 2e-2, which leaves a
large budget):
  * G is shipped to the device as bf16 and the output is returned as bf16
    (host casts back to fp32).  This halves the dominant HBM traffic
    (G + out fall from 64 MiB to 32 MiB per core) and bounds the
    *relative* error of every output element by ~2^-9 per rounding, so
    both the absmax-relative and the floored-elementwise metrics stay
    ~1e-2 or better.
  * scores itself is computed with an fp16 hi/lo split (one K=30 fp16
    matmul per [128,512] chunk; fp16 runs 1 cycle/row like bf16):
        s = sh + sl,  t = th + tl   (hi/lo fp16 pairs)
        lhsT = [sh; 2^-6*sh; 2^6*sl],  rhs = [th; 2^6*tl; 2^-6*th]
    so one accumulation produces sh.th + sh.tl + sl.th exactly (the
    2^+-6 pair scalings are exact and keep every retained term out of
    fp16-subnormal flush range); only the ~2^-22 sl.tl term is dropped.
    Scores error ~2^-21 relative, so the near-zero-scores elementwise
    amplification that dominated the old bf16-split kernel (1.8e-2)
    drops to ~1e-3.

Per-core pipeline, groups of 4 batches (32 batches/core, 4 row-chunks of
128 rows each):
  PE:  4x one K=30 fp16 matmul -> one 4-bank PSUM tile [128, 4, 512]
  ACT: sq = Square(scores)  PSUM->SBUF fp32, one FD=2048 ACTIVATE/batch
  DVE: scalar_tensor_tensor: att = sq*G (bf16), den_col = rowsum (fp32)
  DVE: rec = 1/(den + 0.001) once per 4-batch group (TT-add + reciprocal)
  ACT/DVE 2/2 split: out_chunk = att * rec[:, c] -> bf16 (DVE 4x mode)
  G in / out move as 2 MiB (4-batch) DMAs in the interleaved row layout
  (attention row n = 4p + j at partition p) so each partition's slice is
  4 KiB contiguous in HBM per batch; output DMAs issue from the ACT
  HWDGE ring to avoid head-of-line blocking the G input issues on the
  Sync ring.

Engine budget (measured, final): ACT 141.1us busy (97.9%), DVE 140.9us
(97.7%) -- both saturated and equal; wall 144-149us/core vs the ~105us
HBM stream (34 MiB/core).  225.5us staged baseline -> 148.9us (1.51x).
Errors: absmax-rel 5.6e-3, floored-elementwise 1.11e-2 (gate 2e-2).

NOTE: GPSIMD was tried for the scale pass and is useless here -- its
tensor_scalar with a per-partition AP scalar runs ~9us per [128,512]
chunk, and any DVE 2-port op (tensor_scalar 2x_2p/4x_2p) running
concurrently with a GPSIMD op mutually blocks both engines.
tensor_tensor_reduce crashes the device (NRT unrecoverable); the STT +
separate eps-add is the working formulation.

Sharding: pure data parallel - batch axis 256 split as 32 per core over 8
cores.  Weights are folded into t on the host.
"""

import numpy as np

# Problem shapes (hardcoded per contract)
B_FULL = 256
N = 512
K_IN = 10
HID = 32
N_CORES = 8
B_LOC = B_FULL // N_CORES  # 32
P = 128                    # SBUF partitions per row-chunk
N_CHUNK = N // P           # 4
KOPS = 3 * K_IN            # fp16 split: [sh; 2^-6 sh; 2^6 sl]

# How many of the 4 per-batch final-scale chunks run on ACT (rest on DVE),
# indexed by batch-within-group (ACT/DVE equalize at ~1.75 ACT chunks).
ACT_SCALE_CHUNKS = (2, 2, 2, 1)

_cache = {}


def _build_nc(b_loc=B_LOC):
    import concourse.mybir as mybir
    from concourse import bacc
    from concourse.tile import TileContext
    from contextlib import ExitStack

    f32 = mybir.dt.float32
    f16 = mybir.dt.float16
    bf16 = mybir.dt.bfloat16
    nc = bacc.Bacc("TRN2", target_bir_lowering=False, debug=False,
                   num_devices=N_CORES)

    lhs_d = nc.dram_tensor("lhs", [b_loc, KOPS, N], f16,
                           kind="ExternalInput")
    rhs_d = nc.dram_tensor("rhs", [b_loc, KOPS, N], f16,
                           kind="ExternalInput")
    G_d = nc.dram_tensor("G", [b_loc, N, N], bf16, kind="ExternalInput")
    out_d = nc.dram_tensor("out", [b_loc, N, N], bf16, kind="ExternalOutput")

    # DMA grain: G/out move 4 batches (2 MiB) per DMA; the small fp16
    # operands also move 4 batches per DMA.
    GB = min(4, b_loc)      # batches per G/out DMA and per rec group
    SB = min(4, b_loc)      # batches per lhs/rhs DMA

    with TileContext(nc) as tc, ExitStack() as ctx:
        st_pool = ctx.enter_context(tc.tile_pool(name="st", bufs=2))
        g_pool = ctx.enter_context(tc.tile_pool(name="g", bufs=4))
        sq_pool = ctx.enter_context(tc.tile_pool(name="sq", bufs=3))
        att_pool = ctx.enter_context(tc.tile_pool(name="att", bufs=7))
        out_pool = ctx.enter_context(tc.tile_pool(name="o", bufs=2))
        den_pool = ctx.enter_context(tc.tile_pool(name="den", bufs=3))
        ps_pool = ctx.enter_context(tc.tile_pool(name="ps", bufs=2, space="PSUM"))

        eps_t = den_pool.tile([P, GB, N_CHUNK], f32, tag="eps")
        nc.vector.memset(eps_t, 0.001)

        n_grp = b_loc // GB

        def issue_st(k, split=False):
            """Prefetch the fp16 matmul operands for group k.  The first
            group is issued per-batch (interleaved with per-batch G below)
            so the PE's first matmul starts within ~2us."""
            lhs_t = st_pool.tile([KOPS, SB, N], f16, tag="lhs")
            rhs_t = st_pool.tile([KOPS, SB, N], f16, tag="rhs")
            bb = k * GB
            if split:
                for i in range(SB):
                    nc.sync.dma_start(
                        out=lhs_t[:, i:i + 1, :],
                        in_=lhs_d.ap()[bb + i:bb + i + 1].rearrange(
                            "b k n -> k b n"))
                    nc.sync.dma_start(
                        out=rhs_t[:, i:i + 1, :],
                        in_=rhs_d.ap()[bb + i:bb + i + 1].rearrange(
                            "b k n -> k b n"))
            else:
                nc.sync.dma_start(
                    out=lhs_t,
                    in_=lhs_d.ap()[bb:bb + SB].rearrange("b k n -> k b n"))
                nc.sync.dma_start(
                    out=rhs_t,
                    in_=rhs_d.ap()[bb:bb + SB].rearrange("b k n -> k b n"))
            return lhs_t, rhs_t

        def issue_g(k, split=False):
            """Prefetch G for group k.  Interleaved row layout: attention
            row n = 4p + j lives at partition p, free-slot j, so every
            partition's slice of G_b is 4 KiB contiguous in HBM.  The
            first group is split into per-batch DMAs so batch 0's G lands
            ~4x sooner and the DVE ramps immediately."""
            g_t = g_pool.tile([P, GB, N_CHUNK, N], bf16, tag="G")
            bb = k * GB
            if split:
                for i in range(GB):
                    if i == 0:
                        # Batch 0 lands per-chunk (128 KiB each) so the
                        # first STT starts as soon as possible.
                        for c in range(N_CHUNK):
                            nc.sync.dma_start(
                                out=g_t[:, 0:1, c:c + 1],
                                in_=G_d.ap()[bb:bb + 1].rearrange(
                                    "b (p j) n -> p b j n", p=P)[:, :, c:c + 1])
                        continue
                    nc.sync.dma_start(
                        out=g_t[:, i:i + 1],
                        in_=G_d.ap()[bb + i:bb + i + 1].rearrange(
                            "b (p j) n -> p b j n", p=P))
            else:
                nc.sync.dma_start(
                    out=g_t,
                    in_=G_d.ap()[bb:bb + GB].rearrange(
                        "b (p j) n -> p b j n", p=P))
            return g_t

        # Startup: operands first (PE can start within ~2us), then the
        # first group's G per-batch, then group 1's bulk G.
        st_tiles = {0: issue_st(0, split=True)}
        g_tiles = {0: issue_g(0, split=True), 1: issue_g(1)}

        for k in range(n_grp):
            bb = k * GB
            if k + 1 < n_grp:
                st_tiles[k + 1] = issue_st(k + 1)
            if k + 2 < n_grp:
                g_tiles[k + 2] = issue_g(k + 2)
            lhs_t, rhs_t = st_tiles.pop(k)
            g_t = g_tiles.pop(k)

            o_t = out_pool.tile([P, GB, N_CHUNK, N], bf16, tag="o")
            den_g = den_pool.tile([P, GB, N_CHUNK], f32, tag="den")
            rec_g = den_pool.tile([P, GB, N_CHUNK], f32, tag="rec")

            att_tiles = []
            for i in range(GB):
                # lhsT view: chunk j selects columns n = 4p + j (stride 4)
                # of the [30, 512] stationary operand for this batch.
                lhs_v = lhs_t[:, i, :].rearrange(
                    "k (p j) -> k j p", j=N_CHUNK)
                rhs_b = rhs_t[:, i, :]

                att_t = att_pool.tile([P, N_CHUNK, N], bf16, tag="att")
                att_tiles.append(att_t)

                # All 4 chunks share a 4-bank PSUM tile so the Square runs
                # as a single FD=2048 ACTIVATE.
                ps4 = ps_pool.tile([P, N_CHUNK, N], f32, tag="ps")
                sq4 = sq_pool.tile([P, N_CHUNK, N], f32, tag="sq")
                for c in range(N_CHUNK):
                    nc.tensor.matmul(
                        out=ps4[:, c, :],
                        lhsT=lhs_v[:, c, :],
                        rhs=rhs_b,
                        start=True, stop=True,
                    )
                nc.scalar.activation(
                    out=sq4, in_=ps4,
                    func=mybir.ActivationFunctionType.Square)
                for c in range(N_CHUNK):
                    # att = sq * G ; den = sum(att, axis=-1)
                    nc.vector.scalar_tensor_tensor(
                        out=att_t[:, c, :],
                        in0=sq4[:, c, :],
                        scalar=1.0,
                        in1=g_t[:, i, c, :],
                        op0=mybir.AluOpType.mult,
                        op1=mybir.AluOpType.mult,
                        accum_out=den_g[:, i, c:c + 1],
                    )
                # rec = 1 / (den + 0.001).  Per 2 batches (per batch in the
                # last group) so the scale work unlocks incrementally and
                # ACT never waits on a whole group's worth of STTs.
                if k == n_grp - 1:
                    lo = i
                elif i % 2 == 1:
                    lo = i - 1
                else:
                    continue
                nc.vector.tensor_tensor(
                    out=rec_g[:, lo:i + 1], in0=den_g[:, lo:i + 1],
                    in1=eps_t[:, lo:i + 1], op=mybir.AluOpType.add)
                nc.vector.reciprocal(
                    out=rec_g[:, lo:i + 1], in_=rec_g[:, lo:i + 1])

            for i in range(GB):
                n_act = ACT_SCALE_CHUNKS[i % len(ACT_SCALE_CHUNKS)]
                for c in range(N_CHUNK):
                    if c < n_act:
                        nc.scalar.mul(o_t[:, i, c, :], att_tiles[i][:, c, :],
                                      rec_g[:, i, c:c + 1])
                    else:
                        nc.vector.tensor_scalar_mul(
                            o_t[:, i, c, :], att_tiles[i][:, c, :],
                            rec_g[:, i, c:c + 1])

            # Output DMA issues from the ACT HWDGE ring: a not-yet-ready
            # out(k) must never head-of-line-block the G/operand prefetch
            # stream on the Sync ring (tried: Sync ring went 83% busy on
            # HOL waits and the whole kernel regressed 12%).  The last
            # group goes out per-batch so the tail drain overlaps the
            # final scales.
            if k >= n_grp - 2:
                for i in range(GB):
                    nc.scalar.dma_start(
                        out=out_d.ap()[bb + i:bb + i + 1].rearrange(
                            "b (p j) n -> p b j n", p=P),
                        in_=o_t[:, i:i + 1])
            else:
                nc.scalar.dma_start(
                    out=out_d.ap()[bb:bb + GB].rearrange(
                        "b (p j) n -> p b j n", p=P),
                    in_=o_t)

    nc.compile()
    return nc


def _host_prep(s, Qweight, Kweight):
    """Returns fp16 packed lhs [B,30,N] = [sh; 2^-6 sh; 2^6 sl] and
    rhs [B,30,N] = [th; 2^6 tl; 2^-6 th] so one K=30 fp16 matmul computes
    sh.th + sh.tl + sl.th (pair scalings are exact powers of two and keep
    every term's operands out of fp16-subnormal flush range)."""
    f16 = np.float16
    s = np.asarray(s, dtype=np.float32)
    A = np.asarray(Qweight, np.float64) @ np.asarray(Kweight, np.float64).T
    sT = np.ascontiguousarray(s.transpose(0, 2, 1))          # [B, 10, N]
    t = np.einsum("kl,bln->bkn", A, sT.astype(np.float64)).astype(np.float32)

    sh = sT.astype(f16)
    sl = (sT - sh.astype(np.float32)).astype(f16)
    th = t.astype(f16)
    tl = (t - th.astype(np.float32)).astype(f16)

    sc = np.float32(2.0 ** 6)
    rsc = np.float32(2.0 ** -6)
    lhs = np.concatenate(
        [sh, (sh.astype(np.float32) * rsc).astype(f16),
         (sl.astype(np.float32) * sc).astype(f16)], axis=1)   # [B, 30, N]
    rhs = np.concatenate(
        [th, (tl.astype(np.float32) * sc).astype(f16),
         (th.astype(np.float32) * rsc).astype(f16)], axis=1)  # [B, 30, N]
    return np.ascontiguousarray(lhs), np.ascontiguousarray(rhs)


def _run(in_maps, trace=False, **kw):
    from concourse.bass_utils import run_bass_kernel_spmd
    if "nc" not in _cache:
        _cache["nc"] = _build_nc()
    nc = _cache["nc"]
    return run_bass_kernel_spmd(
        nc, in_maps, core_ids=list(range(N_CORES)), trace=trace, **kw)


def _make_in_maps(s, Gmat, Qweight, Kweight):
    import ml_dtypes
    bf = ml_dtypes.bfloat16
    lhs, rhs = _host_prep(s, Qweight, Kweight)
    Gmat = np.asarray(Gmat, dtype=np.float32).astype(bf)
    in_maps = []
    for c in range(N_CORES):
        sl = slice(c * B_LOC, (c + 1) * B_LOC)
        in_maps.append({
            "lhs": np.ascontiguousarray(lhs[sl]),
            "rhs": np.ascontiguousarray(rhs[sl]),
            "G": np.ascontiguousarray(Gmat[sl]),
        })
    return in_maps


def kernel_traced(s, Gmat, Qweight, Kweight, trace=True):
    """Like kernel() but returns (output, BassKernelResults)."""
    in_maps = _make_in_maps(s, Gmat, Qweight, Kweight)
    res = _run(in_maps, trace=trace)
    out = np.concatenate(
        [np.asarray(r["out"], dtype=np.float32) for r in res.results], axis=0)
    return out, res


def kernel(s, Gmat, Qweight, Kweight):
    out, _ = kernel_traced(s, Gmat, Qweight, Kweight, trace=False)
    return out


# revision 27
# speedup vs baseline: 1.0373x; 1.0373x over previous
"""Trainium2 Bass kernel for nn_Attention_4080218931831 (sparse_attention).

Computes, for each batch b:
    q = s_b @ Qw           [512, 32]
    k = s_b @ Kw           [512, 32]
    scores = q @ k^T       [512, 512]
    att = scores^2 * G_b
    out = att / (sum(att, axis=2, keepdims=True) + 0.001)

Algebraic refactor: scores = s_b @ (Qw @ Kw^T) @ s_b^T = s_b @ t_b where
t_b = A @ s_b^T and A = Qw @ Kw^T is [10, 10].  A and t are precomputed on
the host in float64 (0.06% of total FLOPs); the dominant [512,10]x[10,512]
matmul per batch runs on the PE.

Precision strategy (the harness gate is rel_err < 2e-2, which leaves a
large budget):
  * G is shipped to the device as bf16 and the output is returned as bf16
    (host casts back to fp32).  This halves the dominant HBM traffic
    (G + out fall from 64 MiB to 32 MiB per core) and bounds the
    *relative* error of every output element by ~2^-9 per rounding, so
    both the absmax-relative and the floored-elementwise metrics stay
    ~1e-2 or better.
  * scores itself is computed with an fp16 hi/lo split (one K=30 fp16
    matmul per [128,512] chunk; fp16 runs 1 cycle/row like bf16):
        s = sh + sl,  t = th + tl   (hi/lo fp16 pairs)
        lhsT = [sh; 2^-6*sh; 2^6*sl],  rhs = [th; 2^6*tl; 2^-6*th]
    so one accumulation produces sh.th + sh.tl + sl.th exactly (the
    2^+-6 pair scalings are exact and keep every retained term out of
    fp16-subnormal flush range); only the ~2^-22 sl.tl term is dropped.
    Scores error ~2^-21 relative, so the near-zero-scores elementwise
    amplification that dominated the old bf16-split kernel (1.8e-2)
    drops to ~1e-3.

Per-core pipeline, groups of 4 batches (32 batches/core, 4 row-chunks of
128 rows each):
  PE:  4x one K=30 fp16 matmul -> one 4-bank PSUM tile [128, 4, 512]
  ACT: sq = Square(scores)  PSUM->SBUF fp32, one FD=2048 ACTIVATE/batch
  DVE: scalar_tensor_tensor: att = sq*G (bf16), den_col = rowsum (fp32)
  DVE: rec = 1/(den + 0.001) once per 4-batch group (TT-add + reciprocal)
  ACT/DVE 2/2 split: out_chunk = att * rec[:, c] -> bf16 (DVE 4x mode)
  G in / out move as 2 MiB (4-batch) DMAs in the interleaved row layout
  (attention row n = 4p + j at partition p) so each partition's slice is
  4 KiB contiguous in HBM per batch; output DMAs issue from the ACT
  HWDGE ring to avoid head-of-line blocking the G input issues on the
  Sync ring.

Engine budget (measured, final): ACT 141.1us busy (97.9%), DVE 140.9us
(97.7%) -- both saturated and equal; wall 144-149us/core vs the ~105us
HBM stream (34 MiB/core).  225.5us staged baseline -> 148.9us (1.51x).
Errors: absmax-rel 5.6e-3, floored-elementwise 1.11e-2 (gate 2e-2).

NOTE: GPSIMD was tried for the scale pass and is useless here -- its
tensor_scalar with a per-partition AP scalar runs ~9us per [128,512]
chunk, and any DVE 2-port op (tensor_scalar 2x_2p/4x_2p) running
concurrently with a GPSIMD op mutually blocks both engines.
tensor_tensor_reduce crashes the device (NRT unrecoverable); the STT +
separate eps-add is the working formulation.

Sharding: pure data parallel - batch axis 256 split as 32 per core over 8
cores.  Weights are folded into t on the host.
"""

import numpy as np

# Problem shapes (hardcoded per contract)
B_FULL = 256
N = 512
K_IN = 10
HID = 32
N_CORES = 8
B_LOC = B_FULL // N_CORES  # 32
P = 128                    # SBUF partitions per row-chunk
N_CHUNK = N // P           # 4
KOPS = 3 * K_IN            # fp16 split: [sh; 2^-6 sh; 2^6 sl]

# How many of the 4 per-batch final-scale chunks run on ACT (rest on DVE),
# indexed by batch-within-group (ACT/DVE equalize at ~1.75 ACT chunks).
ACT_SCALE_CHUNKS = (2, 2, 2, 1)

_cache = {}


def _build_nc(b_loc=B_LOC):
    import concourse.mybir as mybir
    from concourse import bacc
    from concourse.tile import TileContext
    from contextlib import ExitStack

    f32 = mybir.dt.float32
    f16 = mybir.dt.float16
    bf16 = mybir.dt.bfloat16
    nc = bacc.Bacc("TRN2", target_bir_lowering=False, debug=False,
                   num_devices=N_CORES)

    lhs_d = nc.dram_tensor("lhs", [b_loc, KOPS, N], f16,
                           kind="ExternalInput")
    rhs_d = nc.dram_tensor("rhs", [b_loc, KOPS, N], f16,
                           kind="ExternalInput")
    G_d = nc.dram_tensor("G", [b_loc, N, N], bf16, kind="ExternalInput")
    out_d = nc.dram_tensor("out", [b_loc, N, N], bf16, kind="ExternalOutput")

    # DMA grain: G/out move 4 batches (2 MiB) per DMA; the small fp16
    # operands also move 4 batches per DMA.
    GB = min(4, b_loc)      # batches per G/out DMA and per rec group
    SB = min(4, b_loc)      # batches per lhs/rhs DMA

    with TileContext(nc) as tc, ExitStack() as ctx:
        st_pool = ctx.enter_context(tc.tile_pool(name="st", bufs=2))
        g_pool = ctx.enter_context(tc.tile_pool(name="g", bufs=4))
        sq_pool = ctx.enter_context(tc.tile_pool(name="sq", bufs=3))
        att_pool = ctx.enter_context(tc.tile_pool(name="att", bufs=6))
        out_pool = ctx.enter_context(tc.tile_pool(name="o", bufs=2))
        den_pool = ctx.enter_context(tc.tile_pool(name="den", bufs=3))
        ps_pool = ctx.enter_context(tc.tile_pool(name="ps", bufs=2, space="PSUM"))

        eps_t = den_pool.tile([P, GB, N_CHUNK], f32, tag="eps")
        nc.vector.memset(eps_t, 0.001)

        n_grp = b_loc // GB

        def issue_st(k, split=False):
            """Prefetch the fp16 matmul operands for group k.  The first
            group is issued per-batch (interleaved with per-batch G below)
            so the PE's first matmul starts within ~2us."""
            lhs_t = st_pool.tile([KOPS, SB, N], f16, tag="lhs")
            rhs_t = st_pool.tile([KOPS, SB, N], f16, tag="rhs")
            bb = k * GB
            if split:
                for i in range(SB):
                    nc.sync.dma_start(
                        out=lhs_t[:, i:i + 1, :],
                        in_=lhs_d.ap()[bb + i:bb + i + 1].rearrange(
                            "b k n -> k b n"))
                    nc.sync.dma_start(
                        out=rhs_t[:, i:i + 1, :],
                        in_=rhs_d.ap()[bb + i:bb + i + 1].rearrange(
                            "b k n -> k b n"))
            else:
                nc.sync.dma_start(
                    out=lhs_t,
                    in_=lhs_d.ap()[bb:bb + SB].rearrange("b k n -> k b n"))
                nc.sync.dma_start(
                    out=rhs_t,
                    in_=rhs_d.ap()[bb:bb + SB].rearrange("b k n -> k b n"))
            return lhs_t, rhs_t

        def issue_g(k, split=False):
            """Prefetch G for group k.  Interleaved row layout: attention
            row n = 4p + j lives at partition p, free-slot j, so every
            partition's slice of G_b is 4 KiB contiguous in HBM.  The
            first group is split into per-batch DMAs so batch 0's G lands
            ~4x sooner and the DVE ramps immediately."""
            g_t = g_pool.tile([P, GB, N_CHUNK, N], bf16, tag="G")
            bb = k * GB
            if split:
                for i in range(GB):
                    nc.sync.dma_start(
                        out=g_t[:, i:i + 1],
                        in_=G_d.ap()[bb + i:bb + i + 1].rearrange(
                            "b (p j) n -> p b j n", p=P))
            else:
                nc.sync.dma_start(
                    out=g_t,
                    in_=G_d.ap()[bb:bb + GB].rearrange(
                        "b (p j) n -> p b j n", p=P))
            return g_t

        # Startup: operands first (PE can start within ~2us), then the
        # first group's G per-batch, then group 1's bulk G.
        st_tiles = {0: issue_st(0, split=True)}
        g_tiles = {0: issue_g(0, split=True), 1: issue_g(1)}

        for k in range(n_grp):
            bb = k * GB
            if k + 1 < n_grp:
                st_tiles[k + 1] = issue_st(k + 1)
            if k + 2 < n_grp:
                g_tiles[k + 2] = issue_g(k + 2)
            lhs_t, rhs_t = st_tiles.pop(k)
            g_t = g_tiles.pop(k)

            o_t = out_pool.tile([P, GB, N_CHUNK, N], bf16, tag="o")
            den_g = den_pool.tile([P, GB, N_CHUNK], f32, tag="den")
            rec_g = den_pool.tile([P, GB, N_CHUNK], f32, tag="rec")

            att_tiles = []
            for i in range(GB):
                # lhsT view: chunk j selects columns n = 4p + j (stride 4)
                # of the [30, 512] stationary operand for this batch.
                lhs_v = lhs_t[:, i, :].rearrange(
                    "k (p j) -> k j p", j=N_CHUNK)
                rhs_b = rhs_t[:, i, :]

                att_t = att_pool.tile([P, N_CHUNK, N], bf16, tag="att")
                att_tiles.append(att_t)

                # All 4 chunks share a 4-bank PSUM tile so the Square runs
                # as a single FD=2048 ACTIVATE.
                ps4 = ps_pool.tile([P, N_CHUNK, N], f32, tag="ps")
                sq4 = sq_pool.tile([P, N_CHUNK, N], f32, tag="sq")
                for c in range(N_CHUNK):
                    nc.tensor.matmul(
                        out=ps4[:, c, :],
                        lhsT=lhs_v[:, c, :],
                        rhs=rhs_b,
                        start=True, stop=True,
                    )
                nc.scalar.activation(
                    out=sq4, in_=ps4,
                    func=mybir.ActivationFunctionType.Square)
                for c in range(N_CHUNK):
                    # att = sq * G ; den = sum(att, axis=-1)
                    nc.vector.scalar_tensor_tensor(
                        out=att_t[:, c, :],
                        in0=sq4[:, c, :],
                        scalar=1.0,
                        in1=g_t[:, i, c, :],
                        op0=mybir.AluOpType.mult,
                        op1=mybir.AluOpType.mult,
                        accum_out=den_g[:, i, c:c + 1],
                    )
                # rec = 1 / (den + 0.001).  Per 2 batches (per batch in the
                # last group) so the scale work unlocks incrementally and
                # ACT never waits on a whole group's worth of STTs.
                if k == n_grp - 1:
                    lo = i
                elif i % 2 == 1:
                    lo = i - 1
                else:
                    continue
                nc.vector.tensor_tensor(
                    out=rec_g[:, lo:i + 1], in0=den_g[:, lo:i + 1],
                    in1=eps_t[:, lo:i + 1], op=mybir.AluOpType.add)
                nc.vector.reciprocal(
                    out=rec_g[:, lo:i + 1], in_=rec_g[:, lo:i + 1])

            for i in range(GB):
                n_act = ACT_SCALE_CHUNKS[i % len(ACT_SCALE_CHUNKS)]
                for c in range(N_CHUNK):
                    if c < n_act:
                        nc.scalar.mul(o_t[:, i, c, :], att_tiles[i][:, c, :],
                                      rec_g[:, i, c:c + 1])
                    else:
                        nc.vector.tensor_scalar_mul(
                            o_t[:, i, c, :], att_tiles[i][:, c, :],
                            rec_g[:, i, c:c + 1])

            # Output DMA issues from the ACT HWDGE ring: a not-yet-ready
            # out(k) must never head-of-line-block the G/operand prefetch
            # stream on the Sync ring (tried: Sync ring went 83% busy on
            # HOL waits and the whole kernel regressed 12%).  The last
            # group goes out per-batch so the tail drain overlaps the
            # final scales.
            if k == n_grp - 1:
                for i in range(GB):
                    nc.scalar.dma_start(
                        out=out_d.ap()[bb + i:bb + i + 1].rearrange(
                            "b (p j) n -> p b j n", p=P),
                        in_=o_t[:, i:i + 1])
            else:
                nc.scalar.dma_start(
                    out=out_d.ap()[bb:bb + GB].rearrange(
                        "b (p j) n -> p b j n", p=P),
                    in_=o_t)

    nc.compile()
    return nc


def _host_prep(s, Qweight, Kweight):
    """Returns fp16 packed lhs [B,30,N] = [sh; 2^-6 sh; 2^6 sl] and
    rhs [B,30,N] = [th; 2^6 tl; 2^-6 th] so one K=30 fp16 matmul computes
    sh.th + sh.tl + sl.th (pair scalings are exact powers of two and keep
    every term's operands out of fp16-subnormal flush range)."""
    f16 = np.float16
    s = np.asarray(s, dtype=np.float32)
    A = np.asarray(Qweight, np.float64) @ np.asarray(Kweight, np.float64).T
    sT = np.ascontiguousarray(s.transpose(0, 2, 1))          # [B, 10, N]
    t = np.einsum("kl,bln->bkn", A, sT.astype(np.float64)).astype(np.float32)

    sh = sT.astype(f16)
    sl = (sT - sh.astype(np.float32)).astype(f16)
    th = t.astype(f16)
    tl = (t - th.astype(np.float32)).astype(f16)

    sc = np.float32(2.0 ** 6)
    rsc = np.float32(2.0 ** -6)
    lhs = np.concatenate(
        [sh, (sh.astype(np.float32) * rsc).astype(f16),
         (sl.astype(np.float32) * sc).astype(f16)], axis=1)   # [B, 30, N]
    rhs = np.concatenate(
        [th, (tl.astype(np.float32) * sc).astype(f16),
         (th.astype(np.float32) * rsc).astype(f16)], axis=1)  # [B, 30, N]
    return np.ascontiguousarray(lhs), np.ascontiguousarray(rhs)


def _run(in_maps, trace=False, **kw):
    from concourse.bass_utils import run_bass_kernel_spmd
    if "nc" not in _cache:
        _cache["nc"] = _build_nc()
    nc = _cache["nc"]
    return run_bass_kernel_spmd(
        nc, in_maps, core_ids=list(range(N_CORES)), trace=trace, **kw)


def _make_in_maps(s, Gmat, Qweight, Kweight):
    import ml_dtypes
    bf = ml_dtypes.bfloat16
    lhs, rhs = _host_prep(s, Qweight, Kweight)
    Gmat = np.asarray(Gmat, dtype=np.float32).astype(bf)
    in_maps = []
    for c in range(N_CORES):
        sl = slice(c * B_LOC, (c + 1) * B_LOC)
        in_maps.append({
            "lhs": np.ascontiguousarray(lhs[sl]),
            "rhs": np.ascontiguousarray(rhs[sl]),
            "G": np.ascontiguousarray(Gmat[sl]),
        })
    return in_maps


def kernel_traced(s, Gmat, Qweight, Kweight, trace=True):
    """Like kernel() but returns (output, BassKernelResults)."""
    in_maps = _make_in_maps(s, Gmat, Qweight, Kweight)
    res = _run(in_maps, trace=trace)
    out = np.concatenate(
        [np.asarray(r["out"], dtype=np.float32) for r in res.results], axis=0)
    return out, res


def kernel(s, Gmat, Qweight, Kweight):
    out, _ = kernel_traced(s, Gmat, Qweight, Kweight, trace=False)
    return out
